# revision 22
# baseline (speedup 1.0000x reference)
"""DLRM dot-interaction kernel for Trainium2 (Bass/Tile), 8-core data parallel.

Computes, for each sample b:
    combined = concat([dense[b], sparse[b]])          # [27, 128]
    C = combined @ combined.T                          # [27, 27] gram
    out[b] = concat([dense[b], triu_flat(C)])          # [506]

Device strategy (per core, S = 4096 samples):
  - Host pre-transposes inputs to X^T layout [D=128, (samples), 27] so the
    contraction dim D sits on SBUF partitions (no on-device transpose).
  - PAIR-PACKED matmuls: 2 adjacent samples form one matmul with
    lhsT = [128, 64] (their X^T blocks + 10 pad cols) and rhs = [128, 54],
    producing a [64, 54] PSUM block whose two diagonal 27x27 blocks are the
    two grams. Halving the PE instruction stream matters: per-sample
    matmuls (8192+ LdW/MM instrs) lose ~3.2us to instruction fetch at
    every 16KB IRAM block boundary.
  - Pairs alternate between PE column groups 0/1 (tile_position (0,0) /
    (0,64)), so each LdWeights overlaps the other group's running matmul.
  - One PSUM bank holds 16 pairs (2 groups x 8 slots of 54 fp32); a single
    [128, 432] eviction per bank (alternating DVE/ACT) moves it to SBUF.
  - Host interleaves samples per chunk so that each of the 4 diagonal-block
    classes (column group x pair member) is a contiguous run of original
    sample indices; the 4 per-chunk output DMAs then write dense runs into
    DRAM laid out [27, S, 27] (gram row-major), and the host-side triu
    gather is 27 contiguous slice copies.
  - Input loads alternate across both HWDGE rings; output stores use
    SWDGE so their HBM write receipts don't stall the input stream.
  - Dense passthrough (output cols 0:128) is assembled on the host.
"""

import os
import sys

import numpy as np

for _p in (
    "/root/.axon_site",
    "/root/.axon_site/_ro/trn_rl_repo",
    "/opt/trn_rl_repo",
):
    if os.path.isdir(_p) and _p not in sys.path:
        sys.path.append(_p)

import concourse.bacc as bacc
import concourse.bass as bass
import concourse.mybir as mybir
import concourse.tile as tile

NF = 27  # combined features (1 dense + 26 sparse)
D = 128  # embedding dim
B = 32768  # batch
NCORES = 8
S = B // NCORES  # samples per core
PW = 2 * NF  # pair width: 54 columns = 2 samples
PAD = 10  # stationary reads 64 cols = 54 + 10 pad

F32 = mybir.dt.float32

# Upper-triangle (incl. diagonal) flattened offsets: row n starts at TOFF[n],
# length 27 - n. Matches np.triu_indices(27) row-major order.
TOFF = np.concatenate([[0], np.cumsum(NF - np.arange(NF))]).astype(np.int64)
NPAIRS = int(TOFF[NF])  # 378
DOUT = D + NPAIRS  # 506

# Tiling: per PSUM bank: 2 column groups x SLOTS pairs of 54 fp32 (<=512).
SLOTS = 8  # pair slots per bank per group -> 432 fp32 of 512
BANK_SAMP = 2 * SLOTS * 2  # 32 samples per bank


def build_nc(s_per_core=S, kb=16, ib=2):
    """Build the per-core Bass program.

    kb: PSUM banks per chunk (chunk = kb * 32 samples)
    ib: banks per input-DMA tile
    """
    c_sz = kb * BANK_SAMP  # samples per chunk
    assert s_per_core % c_sz == 0, (s_per_core, c_sz)
    assert kb % ib == 0
    nchunks = s_per_core // c_sz
    half = c_sz // 2  # pairs per chunk; class stride in original order

    # Bacc (not raw Bass): its compile() pass legalizes multi-wait matmuls
    # (raw Bass emits >1 wait on LdWeights, which walrus codegen rejects).
    nc = bacc.Bacc("TRN2", target_bir_lowering=False, debug=False)
    xt = nc.dram_tensor(
        "xt", [D, s_per_core * NF + PAD], F32, kind="ExternalInput"
    )
    gram = nc.dram_tensor("gram", [NF, s_per_core, NF], F32, kind="ExternalOutput")

    with tile.TileContext(nc) as tc:
        with (
            tc.tile_pool(name="xin", bufs=8) as xin_pool,
            tc.tile_pool(name="gbuf", bufs=2) as gbuf_pool,
            tc.tile_pool(name="ps", bufs=8, space="PSUM") as ps_pool,
        ):
            in_engines = [nc.sync, nc.scalar]
            rr = {"in": 0, "ev": 0}

            for c0 in range(nchunks):
                gbuf = gbuf_pool.tile([128, kb * SLOTS * PW], F32)
                for bi in range(kb // ib):
                    s_base = c0 * c_sz + bi * ib * BANK_SAMP
                    xin = xin_pool.tile([D, ib * BANK_SAMP * NF + PAD], F32)
                    eng = in_engines[rr["in"] % 2]
                    rr["in"] += 1
                    eng.dma_start(
                        out=xin[:],
                        in_=xt[
                            :,
                            s_base * NF : (s_base + ib * BANK_SAMP) * NF + PAD,
                        ],
                    )
                    for bh in range(ib):
                        b = bi * ib + bh
                        ps = ps_pool.tile([128, SLOTS * PW], F32)
                        # pair p (device order) -> (group g, slot s):
                        # p = g*SLOTS + s within this bank
                        for s in range(SLOTS):
                            for g in range(2):
                                loc = (
                                    bh * BANK_SAMP + (g * SLOTS + s) * 2
                                ) * NF
                                nc.tensor.matmul(
                                    ps[64 * g : 64 * g + 64, s * PW : (s + 1) * PW],
                                    xin[:, loc : loc + 64],
                                    xin[:, loc : loc + PW],
                                    start=True,
                                    stop=True,
                                    tile_position=(0, 64 * g),
                                )
                        dst = gbuf[:, b * SLOTS * PW : (b + 1) * SLOTS * PW]
                        if rr["ev"] % 3 < 2:
                            nc.vector.tensor_copy(dst, ps[:])
                        else:
                            nc.scalar.copy(dst, ps[:])
                        rr["ev"] += 1
                # Output: one DMA per class (g, i) = (column group, pair
                # member). Device pair index within chunk: p = b*2*SLOTS +
                # g*SLOTS + s; original sample index o = i*half + g*half/2 +
                # b*SLOTS + s, so each class is one contiguous run of kb*SLOTS
                # original samples and the dst AP collapses to a single run.
                src4 = gbuf[:].rearrange("p (b s w) -> p b s w", b=kb, s=SLOTS)
                for g in range(2):
                    for i in range(2):
                        o_base = c0 * c_sz + i * half + g * (half // 2)
                        nc.gpsimd.dma_start(
                            out=gram[:, o_base : o_base + kb * SLOTS, :],
                            in_=src4[
                                64 * g + 27 * i : 64 * g + 27 * i + NF,
                                :,
                                :,
                                27 * i : 27 * i + NF,
                            ],
                        )
    nc.finalize()  # runs Bacc.compile() (reg alloc, wait legalization)
    return nc


def device_order(s_per_core=S, kb=16):
    """Permutation: device position d holds original sample order[d]."""
    c_sz = kb * BANK_SAMP
    half = c_sz // 2
    order = np.empty(s_per_core, dtype=np.int64)
    for c0 in range(s_per_core // c_sz):
        base = c0 * c_sz
        for b in range(kb):
            for g in range(2):
                for s in range(SLOTS):
                    p = b * 2 * SLOTS + g * SLOTS + s
                    for i in range(2):
                        order[base + 2 * p + i] = (
                            base + i * half + g * (half // 2) + b * SLOTS + s
                        )
    return order


def host_pack_inputs(dense_features, sparse_features):
    """[B,128] + [B,26,128] -> X^T layout [128, B, 27] fp32."""
    bsz = dense_features.shape[0]
    xt = np.empty((D, bsz, NF), dtype=np.float32)
    xt[:, :, 0] = np.asarray(dense_features, dtype=np.float32).T
    xt[:, :, 1:] = np.asarray(sparse_features, dtype=np.float32).transpose(2, 0, 1)
    return xt


_ORDER_CACHE = {}


def host_core_input(xt, c, s_per_core=S, kb=16):
    """Core c's shard in device order, flattened, with the stationary pad."""
    key = (s_per_core, kb)
    if key not in _ORDER_CACHE:
        _ORDER_CACHE[key] = device_order(s_per_core, kb)
    order = _ORDER_CACHE[key]
    shard = xt[:, c * s_per_core : (c + 1) * s_per_core, :][:, order, :]
    flat = np.ascontiguousarray(shard).reshape(D, s_per_core * NF)
    return np.concatenate([flat, np.zeros((D, PAD), dtype=np.float32)], axis=1)


def host_unpack_output(dense_features, gram_t):
    """dense [B,128] + gram_t [27, B, 27] -> [B, 506] (dense ++ triu)."""
    bsz = dense_features.shape[0]
    out = np.empty((bsz, DOUT), dtype=np.float32)
    out[:, :D] = dense_features
    for n in range(NF):
        lo = D + int(TOFF[n])
        out[:, lo : lo + NF - n] = gram_t[n, :, n:]
    return out


_NC_CACHE = {}


def _get_nc():
    key = (S,)
    if key not in _NC_CACHE:
        _NC_CACHE[key] = build_nc(S)
    return _NC_CACHE[key]


def kernel(dense_features, sparse_features):
    from concourse.bass_utils import run_bass_kernel_spmd

    dense_features = np.asarray(dense_features, dtype=np.float32)
    sparse_features = np.asarray(sparse_features, dtype=np.float32)
    xt = host_pack_inputs(dense_features, sparse_features)

    in_maps = [{"xt": host_core_input(xt, c)} for c in range(NCORES)]
    nc = _get_nc()
    res = run_bass_kernel_spmd(nc, in_maps, core_ids=list(range(NCORES)))
    gram_t = np.concatenate([r["gram"] for r in res.results], axis=1)  # [27, B, 27]
    return host_unpack_output(dense_features, gram_t)


# revision 23
# speedup vs baseline: 1.0254x; 1.0254x over previous
"""DLRM dot-interaction kernel for Trainium2 (Bass/Tile), 8-core data parallel.

Computes, for each sample b:
    combined = concat([dense[b], sparse[b]])          # [27, 128]
    C = combined @ combined.T                          # [27, 27] gram
    out[b] = concat([dense[b], triu_flat(C)])          # [506]

Device strategy (per core, S = 4096 samples):
  - Host pre-transposes inputs to X^T layout [D=128, (samples), 27] so the
    contraction dim D sits on SBUF partitions (no on-device transpose).
  - PAIR-PACKED matmuls: 2 adjacent samples form one matmul with
    lhsT = [128, 64] (their X^T blocks + 10 pad cols) and rhs = [128, 54],
    producing a [64, 54] PSUM block whose two diagonal 27x27 blocks are the
    two grams. Halving the PE instruction stream matters: per-sample
    matmuls (8192+ LdW/MM instrs) lose ~3.2us to instruction fetch at
    every 16KB IRAM block boundary.
  - Pairs alternate between PE column groups 0/1 (tile_position (0,0) /
    (0,64)), so each LdWeights overlaps the other group's running matmul.
  - One PSUM bank holds 16 pairs (2 groups x 8 slots of 54 fp32); a single
    [128, 432] eviction per bank (alternating DVE/ACT) moves it to SBUF.
  - Host interleaves samples per chunk so that each of the 4 diagonal-block
    classes (column group x pair member) is a contiguous run of original
    sample indices; the 4 per-chunk output DMAs then write dense runs into
    DRAM laid out [27, S, 27] (gram row-major), and the host-side triu
    gather is 27 contiguous slice copies.
  - Input loads alternate across both HWDGE rings; output stores use
    SWDGE so their HBM write receipts don't stall the input stream.
  - Dense passthrough (output cols 0:128) is assembled on the host.
"""

import os
import sys

import numpy as np

for _p in (
    "/root/.axon_site",
    "/root/.axon_site/_ro/trn_rl_repo",
    "/opt/trn_rl_repo",
):
    if os.path.isdir(_p) and _p not in sys.path:
        sys.path.append(_p)

import concourse.bacc as bacc
import concourse.bass as bass
import concourse.mybir as mybir
import concourse.tile as tile

NF = 27  # combined features (1 dense + 26 sparse)
D = 128  # embedding dim
B = 32768  # batch
NCORES = 8
S = B // NCORES  # samples per core
PW = 2 * NF  # pair width: 54 columns = 2 samples
PAD = 10  # stationary reads 64 cols = 54 + 10 pad

F32 = mybir.dt.float32

# Upper-triangle (incl. diagonal) flattened offsets: row n starts at TOFF[n],
# length 27 - n. Matches np.triu_indices(27) row-major order.
TOFF = np.concatenate([[0], np.cumsum(NF - np.arange(NF))]).astype(np.int64)
NPAIRS = int(TOFF[NF])  # 378
DOUT = D + NPAIRS  # 506

# Tiling: per PSUM bank: 2 column groups x SLOTS pairs of 54 fp32 (<=512).
SLOTS = 8  # pair slots per bank per group -> 432 fp32 of 512
BANK_SAMP = 2 * SLOTS * 2  # 32 samples per bank


def build_nc(s_per_core=S, kb=16, ib=2):
    """Build the per-core Bass program.

    kb: PSUM banks per chunk (chunk = kb * 32 samples)
    ib: banks per input-DMA tile
    """
    c_sz = kb * BANK_SAMP  # samples per chunk
    assert s_per_core % c_sz == 0, (s_per_core, c_sz)
    assert kb % ib == 0
    nchunks = s_per_core // c_sz
    half = c_sz // 2  # pairs per chunk; class stride in original order

    # Bacc (not raw Bass): its compile() pass legalizes multi-wait matmuls
    # (raw Bass emits >1 wait on LdWeights, which walrus codegen rejects).
    nc = bacc.Bacc("TRN2", target_bir_lowering=False, debug=False)
    xt = nc.dram_tensor(
        "xt", [D, s_per_core * NF + PAD], F32, kind="ExternalInput"
    )
    gram = nc.dram_tensor("gram", [NF, s_per_core, NF], F32, kind="ExternalOutput")

    with tile.TileContext(nc) as tc:
        with (
            tc.tile_pool(name="xin", bufs=8) as xin_pool,
            tc.tile_pool(name="gbuf", bufs=2) as gbuf_pool,
            tc.tile_pool(name="ps", bufs=8, space="PSUM") as ps_pool,
        ):
            in_engines = [nc.sync, nc.scalar]
            rr = {"in": 0, "ev": 0}

            for c0 in range(nchunks):
                gbuf = gbuf_pool.tile([128, kb * SLOTS * PW], F32)
                for bi in range(kb // ib):
                    s_base = c0 * c_sz + bi * ib * BANK_SAMP
                    xin = xin_pool.tile([D, ib * BANK_SAMP * NF + PAD], F32)
                    eng = in_engines[rr["in"] % 2]
                    rr["in"] += 1
                    eng.dma_start(
                        out=xin[:],
                        in_=xt[
                            :,
                            s_base * NF : (s_base + ib * BANK_SAMP) * NF + PAD,
                        ],
                    )
                    for bh in range(ib):
                        b = bi * ib + bh
                        ps = ps_pool.tile([128, SLOTS * PW], F32)
                        # pair p (device order) -> (group g, slot s):
                        # p = g*SLOTS + s within this bank
                        for s in range(SLOTS):
                            for g in range(2):
                                loc = (
                                    bh * BANK_SAMP + (g * SLOTS + s) * 2
                                ) * NF
                                nc.tensor.matmul(
                                    ps[64 * g : 64 * g + 64, s * PW : (s + 1) * PW],
                                    xin[:, loc : loc + 64],
                                    xin[:, loc : loc + PW],
                                    start=True,
                                    stop=True,
                                    tile_position=(0, 64 * g),
                                )
                        dst = gbuf[:, b * SLOTS * PW : (b + 1) * SLOTS * PW]
                        if rr["ev"] % 3 < 2:
                            nc.vector.tensor_copy(dst, ps[:])
                        else:
                            nc.scalar.copy(dst, ps[:])
                        rr["ev"] += 1
                # Output: one DMA per class (g, i) = (column group, pair
                # member). Device pair index within chunk: p = b*2*SLOTS +
                # g*SLOTS + s; original sample index o = i*half + g*half/2 +
                # b*SLOTS + s, so each class is one contiguous run of kb*SLOTS
                # original samples and the dst AP collapses to a single run.
                src4 = gbuf[:].rearrange("p (b s w) -> p b s w", b=kb, s=SLOTS)
                for g in range(2):
                    for i in range(2):
                        o_base = c0 * c_sz + i * half + g * (half // 2)
                        eng = in_engines[rr["in"] % 2]
                        rr["in"] += 1
                        eng.dma_start(
                            out=gram[:, o_base : o_base + kb * SLOTS, :],
                            in_=src4[
                                64 * g + 27 * i : 64 * g + 27 * i + NF,
                                :,
                                :,
                                27 * i : 27 * i + NF,
                            ],
                        )
    nc.finalize()  # runs Bacc.compile() (reg alloc, wait legalization)
    return nc


def device_order(s_per_core=S, kb=16):
    """Permutation: device position d holds original sample order[d]."""
    c_sz = kb * BANK_SAMP
    half = c_sz // 2
    order = np.empty(s_per_core, dtype=np.int64)
    for c0 in range(s_per_core // c_sz):
        base = c0 * c_sz
        for b in range(kb):
            for g in range(2):
                for s in range(SLOTS):
                    p = b * 2 * SLOTS + g * SLOTS + s
                    for i in range(2):
                        order[base + 2 * p + i] = (
                            base + i * half + g * (half // 2) + b * SLOTS + s
                        )
    return order


def host_pack_inputs(dense_features, sparse_features):
    """[B,128] + [B,26,128] -> X^T layout [128, B, 27] fp32."""
    bsz = dense_features.shape[0]
    xt = np.empty((D, bsz, NF), dtype=np.float32)
    xt[:, :, 0] = np.asarray(dense_features, dtype=np.float32).T
    xt[:, :, 1:] = np.asarray(sparse_features, dtype=np.float32).transpose(2, 0, 1)
    return xt


_ORDER_CACHE = {}


def host_core_input(xt, c, s_per_core=S, kb=16):
    """Core c's shard in device order, flattened, with the stationary pad."""
    key = (s_per_core, kb)
    if key not in _ORDER_CACHE:
        _ORDER_CACHE[key] = device_order(s_per_core, kb)
    order = _ORDER_CACHE[key]
    shard = xt[:, c * s_per_core : (c + 1) * s_per_core, :][:, order, :]
    flat = np.ascontiguousarray(shard).reshape(D, s_per_core * NF)
    return np.concatenate([flat, np.zeros((D, PAD), dtype=np.float32)], axis=1)


def host_unpack_output(dense_features, gram_t):
    """dense [B,128] + gram_t [27, B, 27] -> [B, 506] (dense ++ triu)."""
    bsz = dense_features.shape[0]
    out = np.empty((bsz, DOUT), dtype=np.float32)
    out[:, :D] = dense_features
    for n in range(NF):
        lo = D + int(TOFF[n])
        out[:, lo : lo + NF - n] = gram_t[n, :, n:]
    return out


_NC_CACHE = {}


def _get_nc():
    key = (S,)
    if key not in _NC_CACHE:
        _NC_CACHE[key] = build_nc(S)
    return _NC_CACHE[key]


def kernel(dense_features, sparse_features):
    from concourse.bass_utils import run_bass_kernel_spmd

    dense_features = np.asarray(dense_features, dtype=np.float32)
    sparse_features = np.asarray(sparse_features, dtype=np.float32)
    xt = host_pack_inputs(dense_features, sparse_features)

    in_maps = [{"xt": host_core_input(xt, c)} for c in range(NCORES)]
    nc = _get_nc()
    res = run_bass_kernel_spmd(nc, in_maps, core_ids=list(range(NCORES)))
    gram_t = np.concatenate([r["gram"] for r in res.results], axis=1)  # [27, B, 27]
    return host_unpack_output(dense_features, gram_t)


# revision 24
# speedup vs baseline: 2.2823x; 2.2257x over previous
"""v5 fallback: per-sample matmuls, 32-col tiling, clean per-group output DMAs."""

import os
import sys

import numpy as np

for _p in (
    "/root/.axon_site",
    "/root/.axon_site/_ro/trn_rl_repo",
    "/opt/trn_rl_repo",
):
    if os.path.isdir(_p) and _p not in sys.path:
        sys.path.append(_p)

import concourse.bacc as bacc
import concourse.mybir as mybir
import concourse.tile as tile

NF = 27
D = 128
B = 32768
NCORES = 8
S = B // NCORES

F32 = mybir.dt.float32

TOFF = np.concatenate([[0], np.cumsum(NF - np.arange(NF))]).astype(np.int64)
NPAIRS = int(TOFF[NF])
DOUT = D + NPAIRS


def build_nc(s_per_core=S, jb=16, kb=8):
    j_tot = jb * kb
    c_sz = 4 * j_tot
    assert s_per_core % c_sz == 0
    nchunks = s_per_core // c_sz
    bank_sz = 4 * jb
    assert kb % 2 == 0

    nc = bacc.Bacc("TRN2", target_bir_lowering=False, debug=False)
    xt = nc.dram_tensor("xt", [D, s_per_core * NF + 5], F32, kind="ExternalInput")
    gram = nc.dram_tensor("gram", [NF, s_per_core, NF], F32, kind="ExternalOutput")

    with tile.TileContext(nc) as tc:
        with (
            tc.tile_pool(name="xin", bufs=8) as xin_pool,
            tc.tile_pool(name="gbuf", bufs=2) as gbuf_pool,
            tc.tile_pool(name="ps", bufs=8, space="PSUM") as ps_pool,
        ):
            in_engines = [nc.sync, nc.scalar]
            rr = [0]
            for c0 in range(nchunks):
                gbuf = gbuf_pool.tile([128, j_tot * NF], F32)
                for b2 in range(kb // 2):
                    s_base = c0 * c_sz + b2 * 2 * bank_sz
                    xin = xin_pool.tile([D, 2 * bank_sz * NF + 5], F32)
                    eng = in_engines[rr[0] % 2]
                    rr[0] += 1
                    eng.dma_start(
                        out=xin[:],
                        in_=xt[:, s_base * NF : (s_base + 2 * bank_sz) * NF + 5],
                    )
                    for bh in range(2):
                        b = b2 * 2 + bh
                        ps = ps_pool.tile([128, jb * NF], F32)
                        for jbi in range(jb):
                            for g in range(4):
                                loc = (bh * bank_sz + g * jb + jbi) * NF
                                nc.tensor.matmul(
                                    ps[
                                        32 * g : 32 * g + 32,
                                        jbi * NF : (jbi + 1) * NF,
                                    ],
                                    xin[:, loc : loc + 32],
                                    xin[:, loc : loc + NF],
                                    start=True,
                                    stop=True,
                                    tile_position=(0, 32 * g),
                                )
                        nc.vector.tensor_copy(
                            gbuf[:, b * jb * NF : (b + 1) * jb * NF], ps[:]
                        )
                base = gram[:, c0 * c_sz : (c0 + 1) * c_sz, :].rearrange(
                    "p (b four j) m -> p b four j m", four=4, j=jb
                )
                for g in range(4):
                    nc.gpsimd.dma_start(
                        out=base[:, :, g],
                        in_=gbuf[32 * g : 32 * g + NF, :],
                    )
    nc.finalize()
    return nc


def host_pack_inputs(dense_features, sparse_features):
    bsz = dense_features.shape[0]
    xt = np.empty((D, bsz, NF), dtype=np.float32)
    xt[:, :, 0] = np.asarray(dense_features, dtype=np.float32).T
    xt[:, :, 1:] = np.asarray(sparse_features, dtype=np.float32).transpose(2, 0, 1)
    return xt


def host_core_input(xt, c, s_per_core=S):
    flat = np.ascontiguousarray(
        xt[:, c * s_per_core : (c + 1) * s_per_core, :]
    ).reshape(D, s_per_core * NF)
    return np.concatenate([flat, np.zeros((D, 5), dtype=np.float32)], axis=1)


def host_unpack_output(dense_features, gram_t):
    bsz = dense_features.shape[0]
    out = np.empty((bsz, DOUT), dtype=np.float32)
    out[:, :D] = dense_features
    for n in range(NF):
        lo = D + int(TOFF[n])
        out[:, lo : lo + NF - n] = gram_t[n, :, n:]
    return out


_NC_CACHE = {}


def _get_nc():
    key = (S,)
    if key not in _NC_CACHE:
        _NC_CACHE[key] = build_nc(S)
    return _NC_CACHE[key]


def kernel(dense_features, sparse_features):
    from concourse.bass_utils import run_bass_kernel_spmd

    dense_features = np.asarray(dense_features, dtype=np.float32)
    sparse_features = np.asarray(sparse_features, dtype=np.float32)
    xt = host_pack_inputs(dense_features, sparse_features)
    in_maps = [{"xt": host_core_input(xt, c)} for c in range(NCORES)]
    nc = _get_nc()
    res = run_bass_kernel_spmd(nc, in_maps, core_ids=list(range(NCORES)))
    gram_t = np.concatenate([r["gram"] for r in res.results], axis=1)
    return host_unpack_output(dense_features, gram_t)
